# revision 8
# baseline (speedup 1.0000x reference)
"""TopK sparse autoencoder (B=4096, D=4096, L=32768, k=64) on 8 Trainium2
NeuronCores.

Strategy: tensor-parallel over the latent dimension (4096 latents per core).

Per core (one SPMD program, different data):
  1. Encode  pre = (x - pre_bias) @ W_enc_slice.T + latent_bias  for all 4096
     rows.  Matmuls run in tf32 ("float32r") with a hi/lo split (3 products:
     hi*hi + hi*lo + lo*hi), recovering ~fp32 precision at full PE speed.
  2. Per 512-wide latent stripe, vector.max extracts the top-8 per row
     ("candidates"); AllGather shares all cores' candidates (512 per row).
  3. Each core extracts the 64th-largest candidate per row (threshold t)
     via 8 rounds of max8 + match_replace.  Because the global top-64 of a
     row can contain at most 8 entries from a 512-wide chunk in all but
     astronomically rare cases, t is exact; the host verifies this per row
     from the same candidate sets and repairs any flagged row exactly.
  4. latents = pre * (pre >= max(t, tiny)) in one DVE op, PE-transposed to
     [latent, batch] layout, and densely decoded against the core's W_dec
     slice (single-pass tf32).  Partial recons are ReduceScattered so each
     core ends with its batch shard of the final recons (+ pre_bias).

The host assembles the full outputs, builds the dense latents from pre and
t, and exactly repairs any row where the candidate sets cannot prove the
threshold (chunk overflow) or where ties at the threshold would select more
than k elements (reference keeps the lowest indices).
"""

import numpy as np

import concourse.bass as bass
import concourse.mybir as mybir
from concourse.tile import TileContext
from concourse.masks import make_identity
from concourse.bass_utils import run_bass_kernel_spmd

F32 = mybir.dt.float32
F32R = mybir.dt.float32r

N_CORES = 8
B = 4096          # batch rows
D = 4096          # input dim
L = 32768         # latents
L_SH = L // N_CORES   # latents per core (4096)
K_TOP = 64

N_LS = L_SH // 512    # l-stripes per core (8)
N_BT = B // 128       # batch tiles (32)
N_KT = D // 128       # contraction tiles (32)
N_DS = D // 512       # d-stripes for decode (8)
B_SH = B // N_CORES   # batch shard per core for recons (512)

TINY = 1e-45          # smallest positive; mask threshold floor


def _tf32_round(a: np.ndarray) -> np.ndarray:
    """Round-to-nearest-even to tf32 (10-bit mantissa), kept in fp32."""
    bits = np.ascontiguousarray(a, dtype=np.float32).view(np.uint32).astype(np.uint64)
    bits = bits + 0x0FFF + ((bits >> np.uint64(13)) & np.uint64(1))
    bits = (bits & np.uint64(0xFFFFE000)).astype(np.uint32)
    return bits.view(np.float32)


def _fix_multi_waits(nc):
    """This walrus build accepts at most one semaphore wait per instruction.
    Split any multi-wait instruction by hoisting extra waits onto same-engine
    NoOps inserted immediately before it (equivalent: the engine executes its
    stream in order)."""
    for f in nc.m.functions:
        for blk in f.blocks:
            new = []
            changed = False
            for ins in blk.instructions:
                si = ins.sync_info
                waits = list(si.on_wait) if si is not None and si.on_wait else []
                if len(waits) > 1:
                    for i, w in enumerate(waits[:-1]):
                        nop = mybir.InstNoOp(name=f"{ins.name}-w{i}")
                        nop.engine = ins.engine
                        nop.sync_info = mybir.SyncInfo(on_wait=[w], on_update=[])
                        new.append(nop)
                    si.on_wait = waits[-1:]
                    changed = True
                new.append(ins)
            if changed:
                blk.instructions = new


def _build():
    nc = bass.Bass("TRN2", target_bir_lowering=False, debug=False,
                   num_devices=N_CORES)

    # Per-core external inputs (transposed / rounded / split on the host).
    xt_hi = nc.dram_tensor("xt_hi", [D, B], F32R, kind="ExternalInput")
    xt_lo = nc.dram_tensor("xt_lo", [D, B], F32R, kind="ExternalInput")
    we_hi = nc.dram_tensor("we_hi", [D, L_SH], F32R, kind="ExternalInput")
    we_lo = nc.dram_tensor("we_lo", [D, L_SH], F32R, kind="ExternalInput")
    lbias = nc.dram_tensor("lbias", [128, L_SH], F32, kind="ExternalInput")
    wdec_t = nc.dram_tensor("wdec_t", [L_SH, D], F32R, kind="ExternalInput")
    pbias = nc.dram_tensor("pbias", [128, D], F32, kind="ExternalInput")

    # Per-core external outputs.
    pre_out = nc.dram_tensor("pre_out", [B, L_SH], F32, kind="ExternalOutput")
    cand_out = nc.dram_tensor("cand_out", [B, K_TOP], F32, kind="ExternalOutput")
    that_out = nc.dram_tensor("that_out", [B, 1], F32, kind="ExternalOutput")
    recons_out = nc.dram_tensor("recons_out", [B_SH, D], F32, kind="ExternalOutput")

    n_rounds = (K_TOP + 7) // 8

    with TileContext(nc) as tc:
        with (
            tc.tile_pool(name="persist", bufs=1) as persist_pool,
            tc.tile_pool(name="dram", bufs=1, space="DRAM") as dram_pool,
        ):
            # DRAM intermediates (dependency-tracked pool tiles).
            pre_int = dram_pool.tile([B, L_SH], F32)
            cand_cc = dram_pool.tile([B, K_TOP], F32)
            cand_all = dram_pool.tile([N_CORES, B, K_TOP], F32, addr_space="Shared")
            latst = dram_pool.tile([L_SH, B], F32R)
            part = dram_pool.tile([B, D], F32)
            rs_out = dram_pool.tile([B_SH, D], F32)

            # thresholds for all 32 batch tiles live in one [128, 32] tile
            that_sb = persist_pool.tile([128, N_BT], F32)

            # ---------------- Phase E: encode + candidate extraction -------
            with (
                tc.tile_pool(name="wenc", bufs=1) as wenc_pool,
                tc.tile_pool(name="xin", bufs=2) as x_pool,
                tc.tile_pool(name="eout", bufs=4) as eout_pool,
                tc.tile_pool(name="epsum", bufs=4, space="PSUM") as epsum_pool,
            ):
                for ls in range(N_LS):
                    lsl = slice(ls * 512, (ls + 1) * 512)
                    wh_t = wenc_pool.tile([128, N_KT, 512], F32R, tag="wh")
                    wl_t = wenc_pool.tile([128, N_KT, 512], F32R, tag="wl")
                    nc.sync.dma_start(wh_t, we_hi[:, lsl].rearrange("(t p) n -> p t n", p=128))
                    nc.sync.dma_start(wl_t, we_lo[:, lsl].rearrange("(t p) n -> p t n", p=128))
                    lb_t = eout_pool.tile([128, 512], F32, tag="lb", bufs=2)
                    nc.sync.dma_start(lb_t, lbias[:, lsl])

                    for bt in range(N_BT):
                        btl = slice(bt * 128, (bt + 1) * 128)
                        xh_t = x_pool.tile([128, N_KT, 128], F32R, tag="xh")
                        xl_t = x_pool.tile([128, N_KT, 128], F32R, tag="xl")
                        nc.sync.dma_start(xh_t, xt_hi[:, btl].rearrange("(t p) b -> p t b", p=128))
                        nc.sync.dma_start(xl_t, xt_lo[:, btl].rearrange("(t p) b -> p t b", p=128))

                        ps = epsum_pool.tile([128, 512], F32, tag="eps", name="eps")
                        for kt in range(N_KT):
                            nc.tensor.matmul(ps, xh_t[:, kt, :], wh_t[:, kt, :],
                                             start=(kt == 0), stop=False)
                        for kt in range(N_KT):
                            nc.tensor.matmul(ps, xh_t[:, kt, :], wl_t[:, kt, :],
                                             start=False, stop=False)
                        for kt in range(N_KT):
                            nc.tensor.matmul(ps, xl_t[:, kt, :], wh_t[:, kt, :],
                                             start=False, stop=(kt == N_KT - 1))

                        pre_t = eout_pool.tile([128, 512], F32, tag="pre")
                        nc.vector.tensor_tensor(out=pre_t, in0=ps, in1=lb_t,
                                                op=mybir.AluOpType.add)
                        nc.sync.dma_start(pre_int[btl, lsl], pre_t)

                        c8 = eout_pool.tile([128, 8], F32, tag="c8")
                        nc.vector.max(out=c8, in_=pre_t)
                        csl = slice(ls * 8, ls * 8 + 8)
                        nc.sync.dma_start(cand_out[btl, csl], c8)
                        nc.sync.dma_start(cand_cc[btl, csl], c8)

            # publish pre (bulk DRAM->DRAM copy; overlaps later phases)
            nc.sync.dma_start(pre_out[:], pre_int[:])

            # ---------------- Phase T: global threshold per row ------------
            nc.gpsimd.collective_compute(
                "AllGather", mybir.AluOpType.bypass,
                replica_groups=[list(range(N_CORES))],
                ins=[cand_cc[:]], outs=[cand_all[:]],
            )
            with tc.tile_pool(name="tpool", bufs=2) as t_pool:
                for bt in range(N_BT):
                    btl = slice(bt * 128, (bt + 1) * 128)
                    cg = t_pool.tile([128, N_CORES, K_TOP], F32, tag="cg")
                    nc.sync.dma_start(cg, cand_all[:, btl, :].rearrange("c p e -> p c e"))
                    cur = cg.rearrange("p c e -> p (c e)")
                    m8 = t_pool.tile([128, 8], F32, tag="m8")
                    for it in range(n_rounds):
                        nc.vector.max(out=m8, in_=cur)
                        if it < n_rounds - 1:
                            nc.vector.match_replace(out=cur, in_to_replace=m8,
                                                    in_values=cur, imm_value=-1e30)
                    kc = (K_TOP - 1) % 8
                    nc.vector.tensor_scalar_max(
                        that_sb[:, bt:bt + 1], m8[:, kc:kc + 1], TINY)
                    nc.sync.dma_start(that_out[btl, :], that_sb[:, bt:bt + 1])

            # ---------------- Phase M: mask + transpose to latst -----------
            with (
                tc.tile_pool(name="mpool", bufs=4) as m_pool,
                tc.tile_pool(name="mpsum", bufs=4, space="PSUM") as m_psum,
            ):
                ident = m_pool.tile([128, 128], F32, tag="ident", bufs=1)
                make_identity(nc, ident)
                ident_r = m_pool.tile([128, 128], F32R, tag="identr", bufs=1)
                nc.vector.tensor_copy(ident_r, ident)
                for bt in range(N_BT):
                    btl = slice(bt * 128, (bt + 1) * 128)
                    for ls in range(N_LS):
                        lsl = slice(ls * 512, (ls + 1) * 512)
                        pr = m_pool.tile([128, 512], F32, tag="mpre")
                        nc.sync.dma_start(pr, pre_int[btl, lsl])
                        lat = m_pool.tile([128, 512], F32R, tag="mlat")
                        nc.vector.scalar_tensor_tensor(
                            out=lat, in0=pr, scalar=that_sb[:, bt:bt + 1], in1=pr,
                            op0=mybir.AluOpType.is_ge, op1=mybir.AluOpType.mult)
                        for q in range(4):
                            pst = m_psum.tile([128, 128], F32R, tag="tps")
                            nc.tensor.transpose(pst, lat[:, q * 128:(q + 1) * 128], ident_r)
                            ltt = m_pool.tile([128, 128], F32R, tag="ltt")
                            nc.vector.tensor_copy(ltt, pst)
                            nc.sync.dma_start(
                                latst[ls * 512 + q * 128: ls * 512 + (q + 1) * 128, btl], ltt)

            # ---------------- Phase D: decode ------------------------------
            with (
                tc.tile_pool(name="wdec", bufs=2) as wd_pool,
                tc.tile_pool(name="dlat", bufs=2) as dl_pool,
                tc.tile_pool(name="dout", bufs=4) as do_pool,
                tc.tile_pool(name="dpsum", bufs=4, space="PSUM") as d_psum,
            ):
                for ds in range(N_DS):
                    dsl = slice(ds * 512, (ds + 1) * 512)
                    wd_t = wd_pool.tile([128, N_KT, 512], F32R, tag="wd")
                    nc.sync.dma_start(wd_t, wdec_t[:, dsl].rearrange("(t p) n -> p t n", p=128))
                    for bt in range(N_BT):
                        btl = slice(bt * 128, (bt + 1) * 128)
                        la_t = dl_pool.tile([128, N_KT, 128], F32R, tag="la")
                        nc.sync.dma_start(la_t, latst[:, btl].rearrange("(t p) b -> p t b", p=128))
                        ps2 = d_psum.tile([128, 512], F32, tag="dps")
                        for kt in range(N_KT):
                            nc.tensor.matmul(ps2, la_t[:, kt, :], wd_t[:, kt, :],
                                             start=(kt == 0), stop=(kt == N_KT - 1))
                        po = do_pool.tile([128, 512], F32, tag="po")
                        nc.vector.tensor_copy(po, ps2)
                        nc.sync.dma_start(part[btl, dsl], po)

            # ---------------- Phase R: reduce-scatter + pre_bias ------------
            nc.gpsimd.collective_compute(
                "ReduceScatter", mybir.AluOpType.add,
                replica_groups=[list(range(N_CORES))],
                ins=[part[:]], outs=[rs_out[:]],
            )
            with tc.tile_pool(name="rpool", bufs=2) as r_pool:
                pb_t = r_pool.tile([128, D], F32, tag="pb", bufs=1)
                nc.sync.dma_start(pb_t, pbias[:])
                for rt in range(B_SH // 128):
                    rtl = slice(rt * 128, (rt + 1) * 128)
                    rc = r_pool.tile([128, D], F32, tag="rc")
                    nc.sync.dma_start(rc, rs_out[rtl, :])
                    nc.vector.tensor_tensor(out=rc, in0=rc, in1=pb_t,
                                            op=mybir.AluOpType.add)
                    nc.sync.dma_start(recons_out[rtl, :], rc)

    _fix_multi_waits(nc)
    return nc


_CACHED_NC = None
_LAST_RES = None


def _get_nc():
    global _CACHED_NC
    if _CACHED_NC is None:
        _CACHED_NC = _build()
    return _CACHED_NC


def _reference_topk_row(pre_row: np.ndarray, k: int):
    """jax.lax.top_k semantics: values descending, ties broken by lower index."""
    # stable argsort of -pre gives descending values w/ ascending index ties
    idx = np.argsort(-pre_row, kind="stable")[:k]
    return pre_row[idx], idx


def kernel(x, W_enc, latent_bias, pre_bias, W_dec, k):
    x = np.asarray(x, dtype=np.float32)
    W_enc = np.asarray(W_enc, dtype=np.float32)
    latent_bias = np.asarray(latent_bias, dtype=np.float32)
    pre_bias = np.asarray(pre_bias, dtype=np.float32)
    W_dec = np.asarray(W_dec, dtype=np.float32)
    k = int(k)
    assert x.shape == (B, D) and W_enc.shape == (L, D) and W_dec.shape == (D, L)
    assert k == K_TOP, f"kernel specialized for k={K_TOP}, got {k}"

    # ---------------- host prep ----------------
    xc = x - pre_bias[None, :]
    xt = np.ascontiguousarray(xc.T)                       # [D, B]
    xt_hi = _tf32_round(xt)
    xt_lo = _tf32_round(xt - xt_hi)

    in_maps = []
    for c in range(N_CORES):
        sl = slice(c * L_SH, (c + 1) * L_SH)
        wet = np.ascontiguousarray(W_enc[sl, :].T)        # [D, L_SH]
        we_hi = _tf32_round(wet)
        we_lo = _tf32_round(wet - we_hi)
        wdt = _tf32_round(np.ascontiguousarray(W_dec[:, sl].T))   # [L_SH, D]
        lb = np.ascontiguousarray(
            np.broadcast_to(latent_bias[sl][None, :], (128, L_SH)))
        pb = np.ascontiguousarray(
            np.broadcast_to(pre_bias[None, :], (128, D)))
        in_maps.append({
            "xt_hi": xt_hi, "xt_lo": xt_lo,
            "we_hi": we_hi, "we_lo": we_lo,
            "lbias": lb, "wdec_t": wdt, "pbias": pb,
        })

    nc = _get_nc()
    import os as _os
    _trace = bool(_os.environ.get("KERNEL_TRACE"))
    res = run_bass_kernel_spmd(nc, in_maps, list(range(N_CORES)), trace=_trace)
    global _LAST_RES
    _LAST_RES = res
    results = res.results

    # ---------------- host assembly ----------------
    pre_full = np.empty((B, L), dtype=np.float32)
    for c in range(N_CORES):
        pre_full[:, c * L_SH:(c + 1) * L_SH] = results[c]["pre_out"]

    cand = np.concatenate([results[c]["cand_out"] for c in range(N_CORES)],
                          axis=1)                          # [B, 512]
    t_dev = results[0]["that_out"][:, 0]                   # threshold used on device

    # Exactness audit from the candidate sets (chunk = 512 latents -> 8 cands):
    # 64th largest of candidates
    t_host = -np.partition(-cand, K_TOP - 1, axis=1)[:, K_TOP - 1]
    chunk_min = cand.reshape(B, L // 512, 8)[:, :, 7]
    overflow = (chunk_min >= t_host[:, None]).any(axis=1)
    n_ge = (cand >= t_host[:, None]).sum(axis=1)
    bad = overflow | (n_ge != K_TOP) | (t_host != t_dev)

    recons = np.concatenate([results[c]["recons_out"] for c in range(N_CORES)],
                            axis=0)                        # [B, D]

    t_mask = np.maximum(t_dev, np.float32(TINY))
    latents_full = np.where(pre_full >= t_mask[:, None],
                            np.maximum(pre_full, np.float32(0)),
                            np.float32(0)).astype(np.float32, copy=False)

    # Exact repair of flagged rows (reference tie/selection semantics).
    for r in np.nonzero(bad)[0]:
        vals, idx = _reference_topk_row(pre_full[r], k)
        row = np.zeros(L, dtype=np.float32)
        row[idx] = np.maximum(vals, 0)
        latents_full[r] = row
        recons[r] = W_dec[:, idx] @ np.maximum(vals, 0) + pre_bias

    return pre_full, latents_full, recons
